# revision 12
# baseline (speedup 1.0000x reference)
"""Trainium2 Bass kernel for the hindcast/forecast LSTM (nn_HFLSTM).

Model (see reference): input proj x0 = relu(W_in @ [xfc; xq] + b_in), LSTM cell
(PyTorch gate order i,f,g,o), 365 teacher-forced steps then 24 autoregressive
steps feeding the linear output back as the xq feature.

Strategy:
  - Data-parallel: batch 512 -> 8 cores x 64. Weights replicated.
  - Per core, the 64-batch is split into 2 independent 32-wide "chains" whose
    time steps interleave so elementwise latency of one chain hides under the
    other chain's matmuls.
  - Feature-major layout everywhere: activations stored transposed
    ([feature partitions, batch free]) so the recurrent matmul needs no
    per-step transposes. Weights are the stationary operand (bf16 -> FWL).
  - gates.T accumulated in PSUM per chain: x-part (precomputed X0) + bias
    (K=1 ones-row matmuls) + h-part, 8 m-tiles of 128 gates each, PyTorch
    gates permuted to [i, f, o, g] tile order.
  - g rows of W/b are pre-doubled on host and ONE Sigmoid activation covers
    all 1024 gates; tanh(g) is reconstructed as 2*sigmoid(2g) - 1 inside the
    fused DVE ops (scalar_tensor_tensor), saving ACT instructions.
  - c stays fp32; h and all matmul operands are bf16.
"""

import sys

for _p in ("/opt/trn_rl_repo",):
    if _p not in sys.path:
        sys.path.insert(0, _p)

import ml_dtypes
import numpy as np

import concourse.bacc as bacc
import concourse.mybir as mybir
from concourse.bass_utils import run_bass_kernel_spmd
from concourse.tile import TileContext

RHO, HOR, B, H, FIN = 365, 24, 512, 256, 15
NCORES = 8
BC = B // NCORES  # 64 batch per core
CH = 2            # chains per core
CW = BC // CH     # 32 chain width
TPAD = 368        # rho steps padded so TPAD*BC % 512 == 0
NX = TPAD * BC    # 23552 padded rho columns
NHOR = HOR * BC   # 1536
FP32 = mybir.dt.float32
BF16 = mybir.dt.bfloat16
AF = mybir.ActivationFunctionType
ALU = mybir.AluOpType
BF16NP = ml_dtypes.bfloat16

# permute PyTorch [i,f,g,o] row-blocks (256 each) into m-tile order
# [i0,i1,f0,f1,o0,o1,g0,g1]
_PERM = np.r_[0:256, 256:512, 768:1024, 512:768]


def _build_program(b_out_val: float):
    nc = bacc.Bacc("TRN2", target_bir_lowering=False, debug=False,
                   num_devices=NCORES)

    xT_d = nc.dram_tensor("xT", [17, NX], BF16, kind="ExternalInput").ap()
    horxT_d = nc.dram_tensor("horxT", [17, NHOR], BF16, kind="ExternalInput").ap()
    wg_d = nc.dram_tensor("wg", [128, 4096], BF16, kind="ExternalInput").ap()
    biasw_d = nc.dram_tensor("biasw", [1, 1024], BF16, kind="ExternalInput").ap()
    winT_d = nc.dram_tensor("winT", [17, 256], BF16, kind="ExternalInput").ap()
    woutT_d = nc.dram_tensor("woutT", [128, 2], BF16, kind="ExternalInput").ap()
    ones_d = nc.dram_tensor("onesw", [1, 512], BF16, kind="ExternalInput").ap()
    eye_d = nc.dram_tensor("eyew", [128, 128], BF16, kind="ExternalInput").ap()
    bout_d = nc.dram_tensor("boutw", [1, 1], FP32, kind="ExternalInput").ap()
    out_d = nc.dram_tensor("out", [1, NHOR], FP32, kind="ExternalOutput").ap()

    RT = 32           # ring capacity in steps (4 chunks)
    NCH = NX // 512   # 46 bulk chunks, 8 steps each
    LEAD = 3

    with TileContext(nc) as tc:
        with tc.tile_pool(name="const", bufs=1) as cp, \
             tc.tile_pool(name="work", bufs=3) as wp:
            xT = cp.tile([17, NX], BF16, tag="xT")
            horxT = cp.tile([17, NHOR], BF16, tag="horxT")
            wg = cp.tile([128, 4096], BF16, tag="wg")
            biasw = cp.tile([1, 1024], BF16, tag="biasw")
            winT = cp.tile([17, 256], BF16, tag="winT")
            woutT = cp.tile([128, 2], BF16, tag="woutT")
            ones = cp.tile([1, 512], BF16, tag="ones")
            eye = cp.tile([128, 128], BF16, tag="eye")
            bout = cp.tile([1, 1], FP32, tag="bout")
            # Gx ring: per (step, chain) slot of 8 m-tiles x 32 batch, bf16
            ring = cp.tile([128, RT * CH, 8, CW], BF16, tag="ring")
            h_t = cp.tile([128, 2, CH, CW], BF16, tag="h")
            c_t = cp.tile([128, 2, CH, CW], FP32, tag="c")
            out_sb = cp.tile([1, NHOR], FP32, tag="out_sb")

            nc.sync.dma_start(out=xT[:, :], in_=xT_d)
            nc.sync.dma_start(out=horxT[:, :], in_=horxT_d)
            nc.sync.dma_start(out=wg[:, :], in_=wg_d)
            nc.sync.dma_start(out=biasw[:, :], in_=biasw_d)
            nc.sync.dma_start(out=winT[:, :], in_=winT_d)
            nc.sync.dma_start(out=woutT[:, :], in_=woutT_d)
            nc.sync.dma_start(out=ones[:, :], in_=ones_d)
            nc.sync.dma_start(out=eye[:, :], in_=eye_d)
            nc.sync.dma_start(out=bout[:, :], in_=bout_d)
            nc.vector.memset(c_t[:, :, :, :], 0.0)

            def emit_cell(g_ap, S, u, t2, TC, c_view, h_view, kj):
                """gates psum -> sigmoid -> c,h update. kj = free elems per
                hidden k-tile (CW for rho chains, BC for merged hor)."""
                nc.scalar.activation(out=S[:, :], in_=g_ap, func=AF.Sigmoid)

                def gsl(i):
                    return S[:, i * 2 * kj:(i + 1) * 2 * kj].rearrange(
                        "p (k j) -> p k j", k=2)
                # u = (sig(2g) - 0.5) * sig(i)   [= 0.5*sig(i)*tanh(g)]
                nc.vector.scalar_tensor_tensor(
                    out=u[:, :, :], in0=gsl(3), scalar=0.5, in1=gsl(0),
                    op0=ALU.subtract, op1=ALU.mult)
                # t2 = sig(f) * c
                nc.vector.tensor_mul(out=t2[:, :, :], in0=gsl(1), in1=c_view)
                # c = 2*u + t2
                nc.vector.scalar_tensor_tensor(
                    out=c_view, in0=u[:, :, :], scalar=2.0, in1=t2[:, :, :],
                    op0=ALU.mult, op1=ALU.add)
                nc.scalar.activation(out=TC[:, :, :], in_=c_view, func=AF.Tanh)
                # h = sig(o) * tanh(c)
                nc.vector.tensor_mul(out=h_view, in0=gsl(2), in1=TC[:, :, :])

            with tc.tile_pool(name="rhops", bufs=2, space="PSUM") as rp:
                x0_of = {}

                def emit_x0(n):
                    """x0 = relu(W_in x + b_in) for bulk chunk n (512 cols)."""
                    x0 = wp.tile([128, 2, 512], BF16, tag="X0c", bufs=2)
                    psx0 = rp.tile([128, 512], FP32, tag="pcb2")
                    psx1 = rp.tile([128, 512], FP32, tag="pcb2")
                    for m, psx in ((0, psx0), (1, psx1)):
                        nc.tensor.matmul(
                            psx[:, :], winT[:, m * 128:(m + 1) * 128],
                            xT[:, n * 512:(n + 1) * 512], start=True, stop=True)
                    nc.scalar.activation(out=x0[:, 0, :], in_=psx0[:, :],
                                         func=AF.Relu)
                    nc.vector.tensor_scalar_max(out=x0[:, 1, :],
                                                in0=psx1[:, :], scalar1=0.0)
                    x0_of[n] = x0

                def emit_bulk_group(n, m):
                    """Gx m-tile for chunk n (8 steps x 64 batch) -> ring."""
                    x0 = x0_of[n]
                    pg = rp.tile([128, 512], FP32, tag="pcb")
                    nc.tensor.matmul(pg[:, :], wg[:, m * 128:(m + 1) * 128],
                                     x0[:, 0, :], start=True, stop=False)
                    nc.tensor.matmul(pg[:, :],
                                     wg[:, 1024 + m * 128:1024 + (m + 1) * 128],
                                     x0[:, 1, :], start=False, stop=False)
                    nc.tensor.matmul(pg[:, :], biasw[:, m * 128:(m + 1) * 128],
                                     ones[:, :], start=False, stop=True)
                    base = ((8 * n) % RT) * CH
                    dst = ring[:, base:base + 16, m, :]
                    srcv = pg[:, :].rearrange("p (s j) -> p s j", s=16)
                    if m % 2 == 0:
                        nc.scalar.activation(out=dst, in_=srcv, func=AF.Copy)
                    else:
                        nc.vector.tensor_copy(out=dst, in_=srcv)

                def emit_h_mms(g, cidx, t):
                    for m in range(8):
                        for k in range(2):
                            nc.tensor.matmul(
                                g[:, m * CW:(m + 1) * CW],
                                wg[:, (2 + k) * 1024 + m * 128:(2 + k) * 1024 + (m + 1) * 128],
                                h_t[:, k, cidx, :],
                                start=False, stop=(m == 7 and k == 1))

                # ---------------- rho phase ----------------
                for n in range(LEAD + 1):
                    emit_x0(n)
                for n in range(LEAD):
                    for m in range(8):
                        emit_bulk_group(n, m)

                g_next = []
                for cidx in range(CH):
                    g = rp.tile([128, 8 * CW], FP32, tag=f"g{cidx}")
                    nc.tensor.matmul(
                        g[:, :].rearrange("p (m j) -> p m j", m=8),
                        eye[:, :], ring[:, cidx, :, :],
                        start=True, stop=True)
                    g_next.append(g)

                for t in range(RHO):
                    # spread bulk production: one m-group per step
                    n_g = t // 8 + LEAD
                    if n_g < NCH:
                        emit_bulk_group(n_g, t % 8)
                    if t % 8 == 6:
                        n_x = (t + 2) // 8 + LEAD
                        if n_x < NCH and n_x not in x0_of:
                            emit_x0(n_x)
                    for cidx in range(CH):
                        g = g_next[cidx]
                        if t + 1 < RHO:
                            gn = rp.tile([128, 8 * CW], FP32, tag=f"g{cidx}")
                            slot = ((t + 1) % RT) * CH + cidx
                            nc.tensor.matmul(
                                gn[:, :].rearrange("p (m j) -> p m j", m=8),
                                eye[:, :], ring[:, slot, :, :],
                                start=True, stop=False)
                            g_next[cidx] = gn
                        if t > 0:
                            emit_h_mms(g, cidx, t)
                        S = wp.tile([128, 8 * CW], FP32, tag=f"S{cidx}")
                        u = wp.tile([128, 2, CW], FP32, tag=f"u{cidx}")
                        t2 = wp.tile([128, 2, CW], FP32, tag=f"t2{cidx}")
                        TC = wp.tile([128, 2, CW], FP32, tag=f"TC{cidx}")
                        emit_cell(g[:, :], S, u, t2, TC,
                                  c_t[:, :, cidx, :], h_t[:, :, cidx, :], CW)
            # ---------------- hor phase (chains merged) ----------------
            with tc.tile_pool(name="horps", bufs=2, space="PSUM") as hp:
                # prev0 = W_out @ h + b_out  (merged over chains)
                pv = hp.tile([1, BC], FP32, tag="prevH")
                for k in range(2):
                    nc.tensor.matmul(pv[:, :], woutT[:, k:k + 1],
                                     h_t[:, k, :, :],
                                     start=(k == 0), stop=(k == 1))
                nc.scalar.activation(out=horxT[0:1, 0:BC], in_=pv[:, :],
                                     func=AF.Identity, bias=bout[:, 0:1])
                for t in range(HOR):
                    x0ps = hp.tile([128, 2, BC], FP32, tag="x0H")
                    for m in range(2):
                        nc.tensor.matmul(
                            x0ps[:, m, :], winT[:, m * 128:(m + 1) * 128],
                            horxT[:, t * BC:(t + 1) * BC],
                            start=(m == 0), stop=(m == 1))
                    X0H = wp.tile([128, 2, BC], BF16, tag="X0H")
                    nc.scalar.activation(out=X0H[:, :, :], in_=x0ps[:, :, :],
                                         func=AF.Relu)
                    g = hp.tile([128, 8 * BC], FP32, tag="gH")
                    for m in range(8):
                        for k in range(2):
                            nc.tensor.matmul(
                                g[:, m * BC:(m + 1) * BC],
                                wg[:, k * 1024 + m * 128:k * 1024 + (m + 1) * 128],
                                X0H[:, k, :],
                                start=(m == 0 and k == 0), stop=False)
                    for m in range(8):
                        nc.tensor.matmul(
                            g[:, m * BC:(m + 1) * BC],
                            biasw[:, m * 128:(m + 1) * 128], ones[:, 0:BC],
                            start=False, stop=False)
                    for m in range(8):
                        for k in range(2):
                            nc.tensor.matmul(
                                g[:, m * BC:(m + 1) * BC],
                                wg[:, (2 + k) * 1024 + m * 128:(2 + k) * 1024 + (m + 1) * 128],
                                h_t[:, k, :, :],
                                start=False, stop=(m == 7 and k == 1))
                    S = wp.tile([128, 8 * BC], FP32, tag="SH")
                    u = wp.tile([128, 2, CH, CW], FP32, tag="uH")
                    t2 = wp.tile([128, 2, CH, CW], FP32, tag="t2H")
                    TC = wp.tile([128, 2, CH, CW], FP32, tag="TCH")
                    uv = u[:, :, :, :].rearrange("p k c j -> p k (c j)")
                    t2v = t2[:, :, :, :].rearrange("p k c j -> p k (c j)")
                    TCv = TC[:, :, :, :].rearrange("p k c j -> p k (c j)")
                    cv = c_t[:, :, :, :].rearrange("p k c j -> p k (c j)")
                    hv = h_t[:, :, :, :].rearrange("p k c j -> p k (c j)")
                    emit_cell(g[:, :], S, uv, t2v, TCv, cv, hv, BC)
                    pv = hp.tile([1, BC], FP32, tag="prevH")
                    for k in range(2):
                        nc.tensor.matmul(pv[:, :], woutT[:, k:k + 1],
                                         h_t[:, k, :, :],
                                         start=(k == 0), stop=(k == 1))
                    nc.scalar.activation(
                        out=out_sb[:, t * BC:(t + 1) * BC], in_=pv[:, :],
                        func=AF.Identity, bias=bout[:, 0:1])
                    if t + 1 < HOR:
                        nc.scalar.activation(
                            out=horxT[0:1, (t + 1) * BC:(t + 2) * BC],
                            in_=pv[:, :], func=AF.Identity, bias=bout[:, 0:1])

            nc.sync.dma_start(out=out_d, in_=out_sb[:, :])
    nc.compile()
    return nc


def _prep_inputs(xfc_rho, xfc_hor, xq_rho, xq_hor,
                 W_in, b_in, W_ih, W_hh, b_ih, b_hh, W_out, b_out):
    """Host-side layout/dtype staging. Returns (shared weight map, per-core maps)."""
    f32 = np.float32
    Wcat = np.concatenate([np.asarray(W_ih, f32), np.asarray(W_hh, f32)],
                          axis=1)[_PERM]  # [1024, 512]
    bias = (np.asarray(b_ih, f32) + np.asarray(b_hh, f32))[_PERM].copy()
    Wcat[768:1024] *= 2.0  # g rows doubled: tanh(g) = 2*sig(2g) - 1
    bias[768:1024] *= 2.0
    wg_np = np.ascontiguousarray(
        Wcat.T.reshape(4, 128, 1024).transpose(1, 0, 2).reshape(128, 4096)
    ).astype(BF16NP)
    bias_np = bias[None, :].astype(BF16NP)

    winT_np = np.zeros((17, 256), f32)
    Wf = np.asarray(W_in, f32)  # [256, 16], col 15 = xq/prev feature
    winT_np[0] = Wf[:, 15]
    winT_np[1:16] = Wf[:, 0:15].T
    winT_np[16] = np.asarray(b_in, f32)
    winT_np = winT_np.astype(BF16NP)

    woutT_np = np.ascontiguousarray(
        np.asarray(W_out, f32).reshape(2, 128).T).astype(BF16NP)
    ones_np = np.ones((1, 512), BF16NP)
    eye_np = np.eye(128, dtype=np.float32).astype(BF16NP)
    b_out_val = float(np.asarray(b_out, f32).reshape(-1)[0])

    X = np.concatenate([np.asarray(xq_rho, f32), np.asarray(xfc_rho, f32)],
                       axis=-1)  # [RHO, B, 16]; col 0 = xq
    HX = np.asarray(xfc_hor, f32)  # [HOR, B, 15]

    shared = {"wg": wg_np, "biasw": bias_np, "winT": winT_np,
              "woutT": woutT_np, "onesw": ones_np, "eyew": eye_np,
              "boutw": np.array([[b_out_val]], f32)}
    in_maps = []
    for c in range(NCORES):
        xs = X[:, c * BC:(c + 1) * BC, :].reshape(RHO * BC, 16)
        xT_np = np.zeros((17, NX), f32)
        xT_np[0:16, 0:RHO * BC] = xs.T
        xT_np[16, :] = 1.0
        hs = HX[:, c * BC:(c + 1) * BC, :].reshape(NHOR, FIN)
        hxT = np.zeros((17, NHOR), f32)
        hxT[1:16] = hs.T
        hxT[16] = 1.0
        m = dict(shared)
        m["xT"] = xT_np.astype(BF16NP)
        m["horxT"] = hxT.astype(BF16NP)
        in_maps.append(m)
    return in_maps, b_out_val


_TRACE = {"trace": False}  # test.py flips this for profiled runs
_LAST_RESULTS = {}


def kernel(xfc_rho, xfc_hor, xq_rho, xq_hor,
           W_in, b_in, W_ih, W_hh, b_ih, b_hh, W_out, b_out):
    in_maps, b_out_val = _prep_inputs(
        xfc_rho, xfc_hor, xq_rho, xq_hor,
        W_in, b_in, W_ih, W_hh, b_ih, b_hh, W_out, b_out)
    nc = _build_program(b_out_val)
    res = run_bass_kernel_spmd(nc, in_maps, core_ids=list(range(NCORES)),
                               trace=_TRACE["trace"])
    _LAST_RESULTS["res"] = res
    out = np.zeros((HOR, B, 1), np.float32)
    for c in range(NCORES):
        o = res.results[c]["out"].reshape(HOR, BC)
        out[:, c * BC:(c + 1) * BC, 0] = o
    return out
